# revision 1
# baseline (speedup 1.0000x reference)
"""Trainium2 Bass kernel for EvaLinearAttention (nn_EvaLinearAttention_40656160424185).

Strategy: data-parallel over batch B=8 across the 8 NeuronCores (one batch
element per core, no collectives).

Per-core math (x: [N, C], N=4097, C=768, H=12, hd=64):
  qkv = x @ qkv_w.T + bias;  rope on q,k (all tokens but CLS)
  kvT_h = sum_n v_h[n]^T k_roped_h[n]            (pass 1, PSUM-accumulated)
  M_h   = kv_h @ proj_w[:, h].T  -> stacked M [C, C]   (tiny mid phase)
  out   = (q_roped / (hd*N)) @ M + proj_b        (pass 2; attn+proj fused)

Layout: token-major tiles of 128 tokens; x transposed on-chip via PE
transposes to feed contraction-over-C matmuls; rope via DVE elementwise ops
with host-prepared cos/sin tables (CLS row = identity, scale folded into q
tables); biases added by DVE with partition-replicated bias tiles. Big
matmuls run as float32r (full PE rate), kv accumulation exact fp32.
"""

import os
import sys

sys.path.insert(0, "/opt/trn_rl_repo")

import numpy as np

import concourse.bass as bass  # noqa: F401  (AP construction)
import concourse.tile as tile
from concourse import bacc, mybir
from concourse.bass_utils import run_bass_kernel_spmd
from concourse.masks import make_identity

F32 = mybir.dt.float32
F32R = mybir.dt.float32r

B = 8
N = 4097
NPAD = 4224  # 33 * 128
NT = NPAD // 128  # 33 token tiles
C = 768
H = 12
HD = 64
KC = C // 128  # 6 contraction chunks
SCALE = 1.0 / (HD * N)

_CACHE = {}


def _build_nc(mm_dtype_r=True):
    WD = F32R if mm_dtype_r else F32
    nc = bacc.Bacc("TRN2", target_bir_lowering=False, debug=False, num_devices=B)

    x = nc.dram_tensor("x", [NPAD, C], WD, kind="ExternalInput")
    wkv_t = nc.dram_tensor("wkv_t", [C, 2 * C], WD, kind="ExternalInput")
    wq_t = nc.dram_tensor("wq_t", [C, C], WD, kind="ExternalInput")
    pw_t = nc.dram_tensor("pw_t", [C, C], WD, kind="ExternalInput")
    vb = nc.dram_tensor("vb", [1, C], F32, kind="ExternalInput")
    qb = nc.dram_tensor("qb", [1, C], F32, kind="ExternalInput")
    pb = nc.dram_tensor("pb", [1, C], F32, kind="ExternalInput")
    # packed rope tables: [ck(64) | ske(32) | sko(32) | cq(64) | sqe(32) | sqo(32)]
    ropes = nc.dram_tensor("ropes", [NPAD, 256], F32, kind="ExternalInput")
    out = nc.dram_tensor("out", [NPAD, C], F32, kind="ExternalOutput")
    qrt_dram = nc.dram_tensor("qrt_scratch", [NT, 128, C], WD)

    with tile.TileContext(nc) as tc:
        with (
            tc.tile_pool(name="const", bufs=1) as const_pool,
            tc.tile_pool(name="wpool", bufs=1) as wpool,
            tc.tile_pool(name="xin", bufs=3) as xin_pool,
            tc.tile_pool(name="rope_in", bufs=2) as rope_pool,
            tc.tile_pool(name="work", bufs=2) as work_pool,
            tc.tile_pool(name="outp", bufs=3) as out_pool,
            tc.tile_pool(name="mm_ps", bufs=5, space="PSUM") as mm_ps_pool,
            tc.tile_pool(name="kvt_ps", bufs=1, space="PSUM") as kvt_ps_pool,
        ):
            # ---- constants / weights resident in SBUF ----
            ident_f = const_pool.tile([128, 128], F32)
            make_identity(nc, ident_f)
            ident = const_pool.tile([128, 128], WD)
            nc.vector.tensor_copy(ident, ident_f)

            prefetched_xt = {}

            # persistent kvT accumulators, one PSUM bank each:
            # kvt_a = head pairs 0..2, kvt_b = pairs 3..5; pair p block at
            # cols (p%3)*128, rows = e of (h even: 0..63 | h odd: 64..127),
            # cols within block = d of same head (diag 64x64 blocks used).
            # layout per bank: [pair0 | pair1 | pair2 | shared junk] x 128 cols
            # pair p lives in tile p//2 at col (p%2)*256; each pair block is
            # [128 useful | 128 junk] cols (junk = v_p^T x neighboring k cols,
            # never read) so the matmul free dim is 256 -> full f32r rate.
            kvt_t = [
                kvt_ps_pool.tile([128, 512], F32, tag="kvt01", name="kvt01"),
                kvt_ps_pool.tile([128, 512], F32, tag="kvt23", name="kvt23"),
                kvt_ps_pool.tile([128, 384], F32, tag="kvt45", name="kvt45"),
            ]

            kvt_sbs = [
                work_pool.tile([128, 512], WD, tag="kvt_sb01", bufs=1, name="kvt_sb01"),
                work_pool.tile([128, 512], WD, tag="kvt_sb23", bufs=1, name="kvt_sb23"),
                work_pool.tile([128, 384], WD, tag="kvt_sb45", bufs=1, name="kvt_sb45"),
            ]

            def transpose_768(src_sb, dst_sb):
                # 6x [128,128] PE transposes packed into two [128,512] psum
                # tiles (4 + 2 chunks), copied out by ScalarE.
                psA = mm_ps_pool.tile([128, 512], WD, tag="mm512")
                for kc in range(4):
                    nc.tensor.transpose(
                        psA[:, kc * 128 : (kc + 1) * 128],
                        src_sb[:, kc * 128 : (kc + 1) * 128],
                        ident,
                    )
                psB = mm_ps_pool.tile([128, 512], WD, tag="mm512")
                for kc in range(2):
                    nc.tensor.transpose(
                        psB[:, kc * 128 : (kc + 1) * 128],
                        src_sb[:, (4 + kc) * 128 : (5 + kc) * 128],
                        ident,
                    )
                nc.scalar.copy(dst_sb[:, 0:256], psA[:, 0:256])
                nc.scalar.copy(dst_sb[:, 256:512], psA[:, 256:512])
                nc.vector.tensor_copy(dst_sb[:, 512:768], psB[:, 0:256])

            def load_transpose_x(t):
                x_sb = xin_pool.tile([128, C], WD, tag="x_sb")
                nc.sync.dma_start(x_sb, x.ap()[t * 128 : (t + 1) * 128, :])
                xt_sb = xin_pool.tile([128, C], WD, tag="xt_sb")
                transpose_768(x_sb, xt_sb)
                return xt_sb

            for _pt in range(3):
                prefetched_xt[_pt] = load_transpose_x(_pt)

            wkv_sb = wpool.tile([128, KC, 2 * C], WD)
            wq_sb = wpool.tile([128, KC, C], WD)
            pw_sb = wpool.tile([128, KC, C], WD)
            wkv_r = wkv_t.ap().rearrange("(kc p) n -> p kc n", p=128)
            wq_r = wq_t.ap().rearrange("(kc p) n -> p kc n", p=128)
            pw_r = pw_t.ap().rearrange("(kc p) n -> p kc n", p=128)
            for g in range(3):
                # per (group, chunk) pieces: dense group g's matmuls dep only
                # on their own 6 small DMAs, so group 0 can start ~4x earlier
                for kc in range(KC):
                    nc.scalar.dma_start(
                        wkv_sb[:, kc, g * 512 : (g + 1) * 512],
                        wkv_r[:, kc, g * 512 : (g + 1) * 512],
                    )
            vb_full = wpool.tile([128, C], F32)
            nc.scalar.dma_start(vb_full, vb.ap().broadcast_to([128, C]))
            qb_full = wpool.tile([128, C], F32)
            nc.scalar.dma_start(qb_full, qb.ap().broadcast_to([128, C]))
            pb_full = wpool.tile([128, C], F32)
            nc.scalar.dma_start(pb_full, pb.ap().broadcast_to([128, C]))

            for g in range(2):
                for kc in range(KC):
                    gsl = slice(g * 512, min((g + 1) * 512, C))
                    nc.gpsimd.dma_start(wq_sb[:, kc, gsl], wq_r[:, kc, gsl])



            def dense_ps(xt_sb, w_sb, cols):
                """x_tile @ W into PSUM; returns list of (psum_tile, col_slice)."""
                res = []
                for g in range((cols + 511) // 512):
                    gs = slice(g * 512, min((g + 1) * 512, cols))
                    glen = gs.stop - gs.start
                    ps = mm_ps_pool.tile([128, 512], F32, tag="mm512")
                    for kc in range(KC):
                        nc.tensor.matmul(
                            ps[:, :glen],
                            xt_sb[:, kc * 128 : (kc + 1) * 128],
                            w_sb[:, kc, gs],
                            start=(kc == 0),
                            stop=(kc == KC - 1),
                        )
                    res.append((ps, gs))
                return res

            def rope(dst, src, c_sb, se_sb, so_sb, tmp1, tmp2, pair_eng=None):
                # dst = src * cos + rot(src) * sin  (pairwise rotation)
                pe_ = pair_eng if pair_eng is not None else nc.vector
                cb = c_sb.unsqueeze(1).broadcast_to([128, H, HD])
                seb = se_sb.unsqueeze(1).broadcast_to([128, H, HD // 2])
                sob = so_sb.unsqueeze(1).broadcast_to([128, H, HD // 2])
                src_h = src.rearrange("p (h d) -> p h d", h=H)
                src_pair = src.rearrange("p (h i two) -> p h i two", h=H, two=2)
                t1_h = tmp1.rearrange("p (h d) -> p h d", h=H)
                t2_pair = tmp2.rearrange("p (h i two) -> p h i two", h=H, two=2)
                nc.vector.tensor_mul(t1_h, src_h, cb)
                pe_.tensor_mul(t2_pair[:, :, :, 0], src_pair[:, :, :, 1], seb)
                pe_.tensor_mul(t2_pair[:, :, :, 1], src_pair[:, :, :, 0], sob)
                nc.vector.tensor_add(dst, tmp1, tmp2)

            # ================= pass 1: k, v -> kvT =================
            # Emission is software-pipelined: tile t's transposes + dense
            # matmuls are emitted BEFORE tile t-1's rope-dependent PE work
            # (kvT matmuls, qr transposes), so the in-order PE stream has
            # work to do while DVE runs the rope chain.
            back_state = {}

            def p1_front(t):
                xt_sb = prefetched_xt.pop(t, None)
                if xt_sb is None:
                    xt_sb = load_transpose_x(t)
                (ps0, _), (ps1, _), (ps2, _) = dense_ps(xt_sb, wkv_sb, 2 * C)
                (qs0, _), (qs1, _) = dense_ps(xt_sb, wq_sb, C)

                # k (cols 0:768) -> SBUF via ScalarE so rope runs SBUF-only
                k_sb = work_pool.tile([128, C], F32, tag="k_sb")
                nc.scalar.copy(k_sb[:, 0:512], ps0)
                nc.scalar.copy(k_sb[:, 512:768], ps1[:, 0:256])
                # v (cols 768:1536) + v_bias: ScalarE copies PSUM out,
                # GpSimd adds the bias in place (keeps DVE free for rope)
                v_sb = work_pool.tile([128, C], WD, tag="v_sb")
                nc.scalar.copy(v_sb[:, 0:256], ps1[:, 256:512])
                nc.scalar.copy(v_sb[:, 256:768], ps2)
                nc.gpsimd.tensor_add(v_sb, v_sb, vb_full)

                qbs = work_pool.tile([128, C], F32, tag="qbs")
                nc.scalar.copy(qbs[:, 0:512], qs0)
                nc.scalar.copy(qbs[:, 512:768], qs1[:, 0:256])
                nc.gpsimd.tensor_add(qbs, qbs, qb_full)

                rp_sb = rope_pool.tile([128, 256], F32, tag="ropes")
                nc.sync.dma_start(rp_sb, ropes.ap()[t * 128 : (t + 1) * 128, :])
                ck_sb, ske_sb, sko_sb = rp_sb[:, 0:64], rp_sb[:, 64:96], rp_sb[:, 96:128]
                cq_sb, sqe_sb, sqo_sb = rp_sb[:, 128:192], rp_sb[:, 192:224], rp_sb[:, 224:256]

                kr_sb = work_pool.tile([128, C], WD, tag="kr")
                t1 = work_pool.tile([128, C], F32, tag="t1", bufs=1)
                t2 = work_pool.tile([128, C], F32, tag="t2", bufs=1)
                rope(kr_sb, k_sb, ck_sb, ske_sb, sko_sb, t1, t2)
                qr_sb = work_pool.tile([128, C], WD, tag="qr")
                t1b = work_pool.tile([128, C], F32, tag="t1b", bufs=1)
                t2b = work_pool.tile([128, C], F32, tag="t2b")
                rope(qr_sb, qbs, cq_sb, sqe_sb, sqo_sb, t1b, t2b, pair_eng=nc.gpsimd)
                back_state[t] = (kr_sb, v_sb, qr_sb)

            def p1_back(t):
                kr_sb, v_sb, qr_sb = back_state.pop(t)
                # kvT pair-matmuls, f32r F=256 (full PE rate): rhs spans
                # [k_pair | 128 junk cols]; junk lands in the spaced region
                # of the accumulator and is never read. start=True clears a
                # whole PSUM bank, so only the first pair touching each bank
                # sets it (banks split at col 512).
                for p in range(KC):
                    dst = kvt_t[p // 2]
                    pc = (p % 2) * 256
                    fd = 128 if p == KC - 1 else 256
                    nc.tensor.matmul(
                        dst[:, pc : pc + fd],
                        v_sb[:, p * 128 : (p + 1) * 128],
                        kr_sb[:, p * 128 : p * 128 + fd],
                        start=(t == 0 and p % 2 == 0),
                        stop=(t == NT - 1 and p % 2 == 1),
                    )
                    if t == NT - 1 and p % 2 == 1:
                        # final tile: copy each accumulator out as soon as its
                        # last pair lands so the M phase overlaps the rest
                        nc.vector.tensor_copy(kvt_sbs[p // 2], kvt_t[p // 2])
                qrt_sb = work_pool.tile([128, C], WD, tag="qrt")
                transpose_768(qr_sb, qrt_sb)
                nc.sync.dma_start(qrt_dram.ap()[t], qrt_sb)

            for t in range(NT + 1):
                if t < NT:
                    p1_front(t)
                if t == 4:
                    # proj weights are first read in the M phase; loading them
                    # here keeps the startup window's DMA bandwidth for x/wkv/wq
                    for kc in range(KC):
                        nc.gpsimd.dma_start(pw_sb[:, kc], pw_r[:, kc])
                if t >= 1:
                    p1_back(t - 1)

            # ================= mid: M = stack_h(kv_h @ P_h^T) =================
            m_sb = wpool.tile([128, KC, C], WD)
            for p in range(KC):
                kvt_sb = kvt_sbs[p // 2]
                pc = (p % 2) * 256
                for g in range(2):
                    gs = slice(g * 512, min((g + 1) * 512, C))
                    glen = gs.stop - gs.start
                    # f32r matmuls need dst partition 0, so the odd head's
                    # row-group-64 matmul lands in its own tile at partition 0
                    ps = mm_ps_pool.tile([128, 512], F32, tag="mm512")
                    nc.tensor.matmul(
                        ps[0:64, :glen],
                        kvt_sb[0:64, pc : pc + 64],
                        pw_sb[0:64, p, gs],
                        start=True,
                        stop=True,
                        tile_position=(0, 0),
                    )
                    ps2 = mm_ps_pool.tile([128, 512], F32, tag="mm512")
                    nc.tensor.matmul(
                        ps2[0:64, :glen],
                        kvt_sb[64:128, pc + 64 : pc + 128],
                        pw_sb[64:128, p, gs],
                        start=True,
                        stop=True,
                        tile_position=(64, 0),
                    )
                    nc.scalar.copy(m_sb[0:64, p, gs], ps[0:64, :glen])
                    nc.scalar.copy(m_sb[64:128, p, gs], ps2[0:64, :glen])

            # ================= pass 2: out = qrT.T @ M + pb =================
            # same emission pipelining as pass 1: tile t's dense matmuls are
            # emitted before tile t-1's bias-adds/store, keeping PE fed
            p2_state = {}

            def p2_front(t):
                qrt_sb = work_pool.tile([128, C], WD, tag="qrt2", bufs=3)
                nc.sync.dma_start(qrt_sb, qrt_dram.ap()[t])
                p2_state[t] = dense_ps(qrt_sb, m_sb, C)

            def p2_back(t):
                (os0, _), (os1, _) = p2_state.pop(t)
                o_sb = out_pool.tile([128, C], F32, tag="o_sb", bufs=3)
                nc.vector.tensor_add(o_sb[:, 0:512], os0, pb_full[:, 0:512])
                nc.vector.tensor_add(
                    o_sb[:, 512:768], os1[:, 0:256], pb_full[:, 512:768]
                )
                nc.gpsimd.dma_start(out.ap()[t * 128 : (t + 1) * 128, :], o_sb)

            for t in range(NT + 1):
                if t < NT:
                    p2_front(t)
                if t >= 1:
                    p2_back(t - 1)

    nc.compile()
    return nc


def _prep_inputs(x, rope, qkv_w, q_bias, v_bias, proj_w, proj_b):
    f = np.float32
    x_pad = np.zeros((B, NPAD, C), f)
    x_pad[:, :N] = x

    sin = rope[:, :HD].astype(f)
    cos = rope[:, HD:].astype(f)
    ck = np.ones((NPAD, HD), f)
    ck[1:N] = cos
    ske = np.zeros((NPAD, HD // 2), f)
    ske[1:N] = -sin[:, 0::2]
    sko = np.zeros((NPAD, HD // 2), f)
    sko[1:N] = sin[:, 1::2]

    wt = np.ascontiguousarray(qkv_w.T.astype(f))  # [C, 3C]
    common = dict(
        wkv_t=np.ascontiguousarray(wt[:, C:]),
        wq_t=np.ascontiguousarray(wt[:, :C]),
        pw_t=np.ascontiguousarray(proj_w.T.astype(f)),
        vb=np.ascontiguousarray(v_bias.astype(f)[None, :]),
        qb=np.ascontiguousarray(q_bias.astype(f)[None, :]),
        pb=np.ascontiguousarray(proj_b.astype(f)[None, :]),
        ropes=np.concatenate(
            [ck, ske, sko, ck * SCALE, ske * SCALE, sko * SCALE], axis=1
        ).astype(f),
    )
    in_maps = []
    for b in range(B):
        m = dict(common)
        m["x"] = np.ascontiguousarray(x_pad[b])
        in_maps.append(m)
    return in_maps


def kernel(x, rope, qkv_w, q_bias, v_bias, proj_w, proj_b, _trace=False):
    x = np.asarray(x, dtype=np.float32)
    rope = np.asarray(rope, dtype=np.float32)
    qkv_w = np.asarray(qkv_w, dtype=np.float32)
    q_bias = np.asarray(q_bias, dtype=np.float32)
    v_bias = np.asarray(v_bias, dtype=np.float32)
    proj_w = np.asarray(proj_w, dtype=np.float32)
    proj_b = np.asarray(proj_b, dtype=np.float32)
    if "nc" not in _CACHE:
        _CACHE["nc"] = _build_nc(mm_dtype_r=os.environ.get("MM_F32R", "1") == "1")
    nc = _CACHE["nc"]
    in_maps = _prep_inputs(x, rope, qkv_w, q_bias, v_bias, proj_w, proj_b)
    res = run_bass_kernel_spmd(nc, in_maps, core_ids=list(range(B)), trace=_trace)
    out = np.stack([res.results[b]["out"][:N] for b in range(B)], axis=0)
    if _trace:
        _CACHE["last_result"] = res
    return out.astype(np.float32)



# revision 5
# speedup vs baseline: 1.1422x; 1.1422x over previous
"""Trainium2 Bass kernel for EvaLinearAttention (nn_EvaLinearAttention_40656160424185).

Strategy: data-parallel over batch B=8 across the 8 NeuronCores (one batch
element per core, no collectives).

Per-core math (x: [N, C], N=4097, C=768, H=12, hd=64):
  qkv = x @ qkv_w.T + bias;  rope on q,k (all tokens but CLS)
  kvT_h = sum_n v_h[n]^T k_roped_h[n]            (pass 1, PSUM-accumulated)
  M_h   = kv_h @ proj_w[:, h].T  -> stacked M [C, C]   (tiny mid phase)
  out   = (q_roped / (hd*N)) @ M + proj_b        (pass 2; attn+proj fused)

Implementation: fp8(e4m3) DoubleRow matmuls for the big qkv projection with
host-side hi/lo error compensation (x = xh+xl exact fp8 pair; W = Wh + Wl,
the xl*Wl cross term dropped). x arrives pre-transposed from the host so no
PE transposes are needed anywhere: k/v come out token-major (for the
token-contracted kvT matmuls) while q is computed directly channel-major
(q^T) via W-stationary DoubleRow matmuls; the rope pair-rotation for q^T
(a cross-partition swap) is realized as a second matmul against a
column-pair-swapped copy of Wq. All on-chip intermediates are bf16; kvT,
M and pass-2 run as plain bf16 matmuls. Scales: x*16, W*32 (fp8 range),
folded back via rope tables (1/512) and proj weights; output is written
bf16 scaled by 2^18 (exact power-of-2, undone on host).
"""

import numpy as np
import ml_dtypes

import concourse.bass as bass  # noqa: F401
import concourse.tile as tile
from concourse import bacc, mybir
from concourse.bass_utils import run_bass_kernel_spmd

F32 = mybir.dt.float32
BF16 = mybir.dt.bfloat16
FP8 = mybir.dt.float8e4
DR = mybir.MatmulPerfMode.DoubleRow

NPF8 = ml_dtypes.float8_e4m3
NPBF = np.dtype(ml_dtypes.bfloat16)

B = 8
N = 4097
NPAD = 4224  # 33 * 128
NT = NPAD // 128
C = 768
H = 12
HD = 64
KC = C // 128  # 6 contraction chunks
NG = 6  # 256-col groups over the 1536 k|v output columns
SW = 32.0  # weight fp8 scale
SX = 16.0  # x fp8 scale
SS = SW * SX  # 512; combined scale carried by qkv psums
OS = 2.0 ** 18  # output scale (exact, undone on host)

_CACHE = {}


def _build_nc():
    nc = bacc.Bacc("TRN2", target_bir_lowering=False, debug=False, num_devices=B)

    x8t = nc.dram_tensor("x8t", [128, NT, KC, 2, 128], FP8, kind="ExternalInput")
    # (hi, hi, lo) packed per (group, chunk) so no 0-stride matmul APs needed
    wkv8 = nc.dram_tensor("wkv8", [128, NG, KC, 3, 256], FP8, kind="ExternalInput")
    wq8 = nc.dram_tensor("wq8", [128, KC, C], FP8, kind="ExternalInput")
    wqr8 = nc.dram_tensor("wqr8", [128, KC, C], FP8, kind="ExternalInput")
    qb8 = nc.dram_tensor("qb8", [1, 2, C], FP8, kind="ExternalInput")
    qbr8 = nc.dram_tensor("qbr8", [1, 2, C], FP8, kind="ExternalInput")
    kropes = nc.dram_tensor("kropes", [NT, 128, 128], F32, kind="ExternalInput")
    qropes = nc.dram_tensor("qropes", [NT, 64, 256], F32, kind="ExternalInput")
    vb512 = nc.dram_tensor("vb512", [1, C], F32, kind="ExternalInput")
    pb18 = nc.dram_tensor("pb18", [1, C], F32, kind="ExternalInput")
    pw_eff = nc.dram_tensor("pw_eff", [128, KC, C], BF16, kind="ExternalInput")
    out = nc.dram_tensor("out", [NPAD, C], BF16, kind="ExternalOutput")

    with tile.TileContext(nc) as tc:
        with (
            tc.tile_pool(name="const", bufs=1) as const_pool,
            tc.tile_pool(name="wpool", bufs=1) as wpool,
            tc.tile_pool(name="qrs", bufs=1) as qrs_pool,
            tc.tile_pool(name="xin", bufs=3) as xin_pool,
            tc.tile_pool(name="tabs", bufs=2) as tab_pool,
            tc.tile_pool(name="work", bufs=2) as work_pool,
            tc.tile_pool(name="outp", bufs=3) as out_pool,
            tc.tile_pool(name="kvps", bufs=1, space="PSUM") as kv_ps_pool,
            tc.tile_pool(name="qqps", bufs=1, space="PSUM") as qq_ps_pool,
            tc.tile_pool(name="kvtps", bufs=1, space="PSUM") as kvt_ps_pool,
        ):
            # ---- constants / weights resident in SBUF ----
            ones8 = const_pool.tile([1, 2, 128], FP8)
            nc.vector.memset(ones8, 0.0)
            nc.vector.memset(ones8[:, 0, :], 1.0)
            qb_sb = const_pool.tile([1, 2, C], FP8)
            nc.scalar.dma_start(qb_sb, qb8.ap())
            qbr_sb = const_pool.tile([1, 2, C], FP8)
            nc.scalar.dma_start(qbr_sb, qbr8.ap())
            vb_full = const_pool.tile([128, C], F32)
            nc.scalar.dma_start(vb_full, vb512.ap().broadcast_to([128, C]))
            pb_full = const_pool.tile([128, C], F32)
            nc.scalar.dma_start(pb_full, pb18.ap().broadcast_to([128, C]))

            wkv_sb = wpool.tile([128, NG, KC, 3, 256], FP8)
            wq_sb = wpool.tile([128, KC, C], FP8)
            wqr_sb = wpool.tile([128, KC, C], FP8)
            pw_sb = wpool.tile([128, KC, C], BF16)
            m_sb = wpool.tile([128, KC, C], BF16)
            # per-group weight DMAs so the first matmuls can start early
            for g in range(NG):
                nc.scalar.dma_start(wkv_sb[:, g], wkv8.ap()[:, g])
            for j in range(KC):
                nc.gpsimd.dma_start(wq_sb[:, j], wq8.ap()[:, j])
                nc.gpsimd.dma_start(wqr_sb[:, j], wqr8.ap()[:, j])

            qrs = qrs_pool.tile([128, NT, C], BF16)

            # persistent kvT accumulator: pairs 0-3 in bank 0 (cols 0:512),
            # pairs 4-5 in bank 1 (cols 512:768, rest junk)
            kvt_ps = kvt_ps_pool.tile([128, 1024], F32, tag="kvt", name="kvt")

            state = {}

            def p1_front(t):
                x_sb = xin_pool.tile([128, KC, 2, 128], FP8, tag="x8t")
                nc.sync.dma_start(x_sb, x8t.ap()[:, t])
                ktab = tab_pool.tile([128, 128], F32, tag="ktab")
                nc.scalar.dma_start(ktab, kropes.ap()[t])
                qtab = tab_pool.tile([128, 256], F32, tag="qtab")
                nc.scalar.dma_start(qtab[0:64, :], qropes.ap()[t])
                nc.scalar.dma_start(qtab[64:128, :], qropes.ap()[t])

                # ---- k|v: out[tok, col] += sum_c x^T[c,:].T @ Wkv[c, col]
                # DoubleRow slots: (xh_c, xl_c) x (Wh_c, Wh_c)  [exact x]
                # then (xh_c, xh_c+1) x (Wl_c, Wl_c+1)          [W residual]
                kv_ps = kv_ps_pool.tile([128, 1536], F32, tag="kv")
                for g in range(NG):
                    dst = kv_ps[:, g * 256 : (g + 1) * 256]
                    for c in range(KC):
                        nc.tensor.matmul(
                            dst,
                            x_sb[:, c, :, :],
                            wkv_sb[:, g, c, 0:2, :],
                            start=(g % 2 == 0 and c == 0),
                            stop=False,
                            perf_mode=DR,
                        )
                    for cp in range(3):
                        c = 2 * cp
                        nc.tensor.matmul(
                            dst,
                            x_sb[:, c : c + 2, 0, :],
                            wkv_sb[:, g, c : c + 2, 2, :],
                            start=False,
                            stop=(cp == 2),
                            perf_mode=DR,
                        )

                # ---- q^T / qrot^T: out[cq, tok] += Wq[c, cq].T @ x^T[c, tok]
                # DoubleRow slots pair adjacent chunks (hi parts only);
                # bias added via a K=1 rank-1 DR matmul.
                qq_ps = qq_ps_pool.tile([128, 1536], F32, tag="qq")
                for side, (w_sb, b_sb, base) in enumerate(
                    ((wq_sb, qb_sb, 0), (wqr_sb, qbr_sb, C))
                ):
                    for m in range(KC):
                        col = base + m * 128
                        dst = qq_ps[:, col : col + 128]
                        first = col in (0, 512, 1024)
                        for j in range(3):
                            nc.tensor.matmul(
                                dst,
                                w_sb[:, 2 * j : 2 * j + 2, m * 128 : (m + 1) * 128],
                                x_sb[:, 2 * j : 2 * j + 2, 0, :],
                                start=(first and j == 0),
                                stop=False,
                                perf_mode=DR,
                            )
                        nc.tensor.matmul(
                            dst,
                            b_sb[:, :, m * 128 : (m + 1) * 128],
                            ones8,
                            start=False,
                            stop=True,
                            perf_mode=DR,
                        )
                state[t] = (kv_ps, qq_ps, ktab, qtab)

            def p1_back(t):
                kv_ps, qq_ps, ktab, qtab = state.pop(t)
                # k-side rope (token-major); tables carry 1/SS
                ck = ktab[:, 0:64].unsqueeze(1).broadcast_to([128, H, 64])
                ske = ktab[:, 64:96].unsqueeze(1).broadcast_to([128, H, 32])
                sko = ktab[:, 96:128].unsqueeze(1).broadcast_to([128, H, 32])
                ksrc = kv_ps[:, 0:768]
                k1 = work_pool.tile([128, C], BF16, tag="k1")
                nc.vector.tensor_mul(
                    k1.rearrange("p (h d) -> p h d", h=H),
                    ksrc.rearrange("p (h d) -> p h d", h=H),
                    ck,
                )
                k2 = work_pool.tile([128, C], BF16, tag="k2")
                k2p = k2.rearrange("p (h i two) -> p h i two", h=H, two=2)
                ksp = ksrc.rearrange("p (h i two) -> p h i two", h=H, two=2)
                nc.vector.tensor_mul(k2p[:, :, :, 0], ksp[:, :, :, 1], ske)
                nc.vector.tensor_mul(k2p[:, :, :, 1], ksp[:, :, :, 0], sko)
                # v eviction (ACT; GPSIMD cannot read PSUM) + bias on Pool
                v_sb = work_pool.tile([128, C], BF16, tag="v")
                nc.scalar.copy(v_sb, kv_ps[:, 768:1536])
                nc.gpsimd.tensor_add(v_sb, v_sb, vb_full)
                # q^T rope muls (channel-major; same table for all 6 chunks)
                cq = qtab[:, 0:128].unsqueeze(1).broadcast_to([128, KC, 128])
                sq = qtab[:, 128:256].unsqueeze(1).broadcast_to([128, KC, 128])
                q1 = work_pool.tile([128, C], BF16, tag="q1")
                nc.vector.tensor_mul(
                    q1.rearrange("p (j n) -> p j n", j=KC),
                    qq_ps[:, 0:768].rearrange("p (j n) -> p j n", j=KC),
                    cq,
                )
                q2 = work_pool.tile([128, C], BF16, tag="q2")
                nc.vector.tensor_mul(
                    q2.rearrange("p (j n) -> p j n", j=KC),
                    qq_ps[:, 768:1536].rearrange("p (j n) -> p j n", j=KC),
                    sq,
                )
                nc.gpsimd.tensor_add(qrs[:, t, :], q1, q2)
                # kvT accumulation (bf16, contraction over the 128 tokens)
                for p in range(KC):
                    sl = slice(p * 128, (p + 1) * 128)
                    for ki, ksrc_sb in enumerate((k1, k2)):
                        nc.tensor.matmul(
                            kvt_ps[:, sl],
                            v_sb[:, sl],
                            ksrc_sb[:, sl],
                            start=(t == 0 and ki == 0 and p in (0, 4)),
                            stop=(t == NT - 1 and ki == 1 and p in (3, 5)),
                        )

            for t in range(NT + 1):
                if t < NT:
                    p1_front(t)
                if t == 4:
                    for j in range(KC):
                        nc.gpsimd.dma_start(pw_sb[:, j], pw_eff.ap()[:, j])
                if t >= 1:
                    p1_back(t - 1)

            # ---- mid: M[d, c] = sum_e kv[h, d, e] * pw_eff[(h,e), c] ----
            kvt_sb = wpool.tile([128, C], BF16)
            nc.vector.tensor_copy(kvt_sb, kvt_ps[:, 0:768])
            kvm = kv_ps_pool.tile([128, 1536], F32, tag="kv")
            qqm = qq_ps_pool.tile([128, 1536], F32, tag="qq")
            for p in range(KC):
                slot = (kvm, qqm)[p % 2][:, 0:768]
                for gi in range(3):
                    gs = slice(gi * 256, (gi + 1) * 256)
                    # pending-zero from start=True covers only the matmul's
                    # own partitions, so each head clears its bank itself
                    st = gi % 2 == 0
                    sp = gi % 2 == 1 or gi == 2
                    nc.tensor.matmul(
                        slot[0:64, gs],
                        kvt_sb[0:64, p * 128 : p * 128 + 64],
                        pw_sb[0:64, p, gs],
                        start=st,
                        stop=sp,
                        tile_position=(0, 0),
                    )
                    nc.tensor.matmul(
                        slot[64:128, gs],
                        kvt_sb[64:128, p * 128 + 64 : p * 128 + 128],
                        pw_sb[64:128, p, gs],
                        start=st,
                        stop=sp,
                        tile_position=(64, 64),
                    )
                nc.scalar.copy(m_sb[:, p, 0:512], slot[:, 0:512])
                nc.scalar.copy(m_sb[:, p, 512:768], slot[:, 512:768])

            # ---- pass 2: out[tok, c] = qr^T.T @ M + pb  (bf16) ----
            p2_state = {}

            def p2_front(t):
                slot = (kvm, qqm, kvt_ps)[t % 3][:, 0:768]
                for gofs, glen in ((0, 512), (512, 256)):
                    dst = slot[:, gofs : gofs + glen]
                    for j in range(KC):
                        nc.tensor.matmul(
                            dst,
                            qrs[:, t, j * 128 : (j + 1) * 128],
                            m_sb[:, j, gofs : gofs + glen],
                            start=(j == 0),
                            stop=(j == KC - 1),
                        )
                p2_state[t] = slot

            def p2_back(t):
                slot = p2_state.pop(t)
                o_sb = out_pool.tile([128, C], BF16, tag="osb")
                nc.vector.tensor_add(o_sb, slot, pb_full)
                nc.gpsimd.dma_start(out.ap()[t * 128 : (t + 1) * 128, :], o_sb)

            for t in range(NT + 1):
                if t < NT:
                    p2_front(t)
                if t >= 1:
                    p2_back(t - 1)

    nc.compile()
    return nc


def _prep_inputs(x, rope, qkv_w, q_bias, v_bias, proj_w, proj_b):
    f = np.float32

    sin = rope[:, :HD].astype(f)
    cos = rope[:, HD:].astype(f)
    cfull = np.zeros((NPAD, HD), f)
    cfull[0] = 1.0
    cfull[1:N] = cos
    sfull = np.zeros((NPAD, HD), f)
    sfull[1:N] = sin

    # k tables (token-major): ck | ske | sko, all carrying 1/SS
    kro = np.zeros((NPAD, 128), f)
    kro[:, 0:64] = cfull / SS
    kro[:, 64:96] = -sfull[:, 0::2] / SS
    kro[:, 96:128] = sfull[:, 1::2] / SS
    kropes = np.ascontiguousarray(kro.reshape(NT, 128, 128))

    # q tables (channel-major, transposed): cq^T | sq_signed^T
    sgn = np.tile(np.array([-1.0, 1.0], f), HD // 2)
    qro = np.zeros((NT, 64, 256), f)
    for t in range(NT):
        qro[t, :, 0:128] = cfull[t * 128 : (t + 1) * 128].T / SS
        qro[t, :, 128:256] = (sfull[t * 128 : (t + 1) * 128] * sgn[None, :HD]).T / SS

    wt = np.ascontiguousarray(qkv_w.T.astype(f))  # [C, 3C]
    Wq, Wkv = wt[:, :C], wt[:, C:]
    perm = np.arange(C).reshape(-1, 2)[:, ::-1].reshape(-1)

    kvh = (Wkv * SW).astype(NPF8)
    kvl = ((Wkv * SW) - kvh.astype(f)).astype(NPF8)
    # [128, NG, KC, 3, 256]: (hi, hi, lo)
    wkv8 = np.empty((128, NG, KC, 3, 256), NPF8)
    hi4 = kvh.reshape(KC, 128, NG, 256)
    lo4 = kvl.reshape(KC, 128, NG, 256)
    wkv8[:, :, :, 0, :] = hi4.transpose(1, 2, 0, 3)
    wkv8[:, :, :, 1, :] = hi4.transpose(1, 2, 0, 3)
    wkv8[:, :, :, 2, :] = lo4.transpose(1, 2, 0, 3)

    def wq_pack(W):
        w8 = (W * SW).astype(NPF8)
        return np.ascontiguousarray(w8.reshape(KC, 128, C).transpose(1, 0, 2))

    qb2 = np.zeros((1, 2, C), NPF8)
    qb2[0, 0] = (q_bias.astype(f) * SS).astype(NPF8)
    qbr2 = np.zeros((1, 2, C), NPF8)
    qbr2[0, 0] = (q_bias.astype(f)[perm] * SS).astype(NPF8)

    pw = proj_w.T.astype(f) * (OS / (HD * N) / SS)
    pw_eff = np.ascontiguousarray(
        pw.reshape(KC, 128, C).transpose(1, 0, 2).astype(NPBF)
    )

    common = dict(
        wkv8=np.ascontiguousarray(wkv8),
        wq8=wq_pack(Wq),
        wqr8=wq_pack(Wq[:, perm]),
        qb8=qb2,
        qbr8=qbr2,
        kropes=kropes,
        qropes=qro,
        vb512=np.ascontiguousarray(v_bias.astype(f)[None, :] * SS),
        pb18=np.ascontiguousarray(proj_b.astype(f)[None, :] * OS),
        pw_eff=pw_eff,
    )

    in_maps = []
    for b in range(B):
        xs = np.zeros((NPAD, C), f)
        xs[:N] = x[b] * SX
        xh = xs.astype(NPF8)
        xl = (xs - xh.astype(f)).astype(NPF8)
        stacked = np.stack([xh, xl], axis=0).reshape(2, NT, 128, KC, 128)
        m = dict(common)
        m["x8t"] = np.ascontiguousarray(stacked.transpose(4, 1, 3, 0, 2))
        in_maps.append(m)
    return in_maps


def kernel(x, rope, qkv_w, q_bias, v_bias, proj_w, proj_b, _trace=False):
    x = np.asarray(x, dtype=np.float32)
    rope = np.asarray(rope, dtype=np.float32)
    qkv_w = np.asarray(qkv_w, dtype=np.float32)
    q_bias = np.asarray(q_bias, dtype=np.float32)
    v_bias = np.asarray(v_bias, dtype=np.float32)
    proj_w = np.asarray(proj_w, dtype=np.float32)
    proj_b = np.asarray(proj_b, dtype=np.float32)
    if "nc" not in _CACHE:
        _CACHE["nc"] = _build_nc()
    nc = _CACHE["nc"]
    in_maps = _prep_inputs(x, rope, qkv_w, q_bias, v_bias, proj_w, proj_b)
    res = run_bass_kernel_spmd(nc, in_maps, core_ids=list(range(B)), trace=_trace)
    out = np.stack(
        [res.results[b]["out"][:N].astype(np.float32) for b in range(B)], axis=0
    )
    if _trace:
        _CACHE["last_result"] = res
    return out * np.float32(1.0 / OS)


# revision 16
# speedup vs baseline: 1.2451x; 1.0901x over previous
"""Trainium2 Bass kernel for EvaLinearAttention (nn_EvaLinearAttention_40656160424185).

Strategy: data-parallel over batch B=8 across the 8 NeuronCores (one batch
element per core, no collectives).

Per-core math (x: [N, C], N=4097, C=768, H=12, hd=64):
  qkv = x @ qkv_w.T + bias;  rope on q,k (all tokens but CLS)
  kvT_h = sum_n v_h[n]^T k_roped_h[n]            (pass 1, PSUM-accumulated)
  M_h   = kv_h @ proj_w[:, h].T  -> stacked M [C, C]   (tiny mid phase)
  out   = (q_roped / (hd*N)) @ M + proj_b        (pass 2; attn+proj fused)

Implementation: fp8(e4m3) DoubleRow matmuls for the big qkv projection with
host-side hi/lo error compensation (x = xh+xl exact fp8 pair; W = Wh + Wl,
the xl*Wl cross term dropped). x arrives pre-transposed from the host so no
PE transposes are needed anywhere: k/v come out token-major (for the
token-contracted kvT matmuls) while q is computed directly channel-major
(q^T) via W-stationary DoubleRow matmuls; the rope pair-rotation for q^T
(a cross-partition swap) is realized as a second matmul against a
column-pair-swapped copy of Wq. All on-chip intermediates are bf16; kvT,
M and pass-2 run as plain bf16 matmuls. Scales: x*16, W*32 (fp8 range),
folded back via rope tables (1/512) and proj weights; output is written
bf16 scaled by 2^18 (exact power-of-2, undone on host).
"""

import numpy as np
import ml_dtypes

import concourse.bass as bass  # noqa: F401
import concourse.tile as tile
from concourse import bacc, mybir
from concourse.bass_utils import run_bass_kernel_spmd

F32 = mybir.dt.float32
BF16 = mybir.dt.bfloat16
FP8 = mybir.dt.float8e4
DR = mybir.MatmulPerfMode.DoubleRow

NPF8 = ml_dtypes.float8_e4m3
NPBF = np.dtype(ml_dtypes.bfloat16)

B = 8
N = 4097
NPAD = 4224  # 33 * 128
NT = NPAD // 128
C = 768
H = 12
HD = 64
KC = C // 128  # 6 contraction chunks
NG = 3  # 512-col groups over the 1536 k|v output columns
SW = 32.0  # weight fp8 scale
SX = 16.0  # x fp8 scale
SS = SW * SX  # 512; combined scale carried by qkv psums
OS = 2.0 ** 18  # output scale (exact, undone on host)

_CACHE = {}


def _build_nc():
    nc = bacc.Bacc("TRN2", target_bir_lowering=False, debug=False, num_devices=B)

    x8t = nc.dram_tensor("x8t", [128, NT, KC, 2, 128], FP8, kind="ExternalInput")
    # (hi, hi, lo) packed per (group, chunk) so no 0-stride matmul APs needed
    wkv8 = nc.dram_tensor("wkv8", [128, NG, KC, 3, 512], FP8, kind="ExternalInput")
    wq8 = nc.dram_tensor("wq8", [128, KC, C], FP8, kind="ExternalInput")
    qbrope = nc.dram_tensor("qbrope", [NT, 128, C], BF16, kind="ExternalInput")
    kropes = nc.dram_tensor("kropes", [NT, 128, 128], BF16, kind="ExternalInput")
    qropes = nc.dram_tensor("qropes", [NT, 64, 256], BF16, kind="ExternalInput")
    vb512 = nc.dram_tensor("vb512", [1, C], F32, kind="ExternalInput")
    pb18 = nc.dram_tensor("pb18", [1, C], F32, kind="ExternalInput")
    pw_eff = nc.dram_tensor("pw_eff", [128, KC, C], BF16, kind="ExternalInput")
    out = nc.dram_tensor("out", [NPAD, C], BF16, kind="ExternalOutput")

    with tile.TileContext(nc) as tc:
        with (
            tc.tile_pool(name="const", bufs=1) as const_pool,
            tc.tile_pool(name="wpool", bufs=1) as wpool,
            tc.tile_pool(name="qrs", bufs=1) as qrs_pool,
            tc.tile_pool(name="xin", bufs=3) as xin_pool,
            tc.tile_pool(name="tabs", bufs=2) as tab_pool,
            tc.tile_pool(name="work", bufs=2) as work_pool,
            tc.tile_pool(name="outp", bufs=3) as out_pool,
            tc.tile_pool(name="kvps", bufs=1, space="PSUM") as kv_ps_pool,
            tc.tile_pool(name="qqps", bufs=1, space="PSUM") as qq_ps_pool,
            tc.tile_pool(name="kvtps", bufs=1, space="PSUM") as kvt_ps_pool,
        ):
            # ---- constants / weights resident in SBUF ----
            wkv_sb = wpool.tile([128, NG, KC, 3, 512], FP8)
            wq_sb = wpool.tile([128, KC, C], FP8)
            pw_sb = wpool.tile([128, KC, C], BF16)
            m_sb = wpool.tile([128, KC, C], BF16)

            qrs = qrs_pool.tile([128, NT, C], BF16)

            # x tiles for the first few iterations are prefetched ahead of
            # the weight DMAs sharing the sync queue
            prefetched_x = {}

            def load_x(t):
                x_sb = xin_pool.tile([128, KC, 2, 128], FP8, tag="x8t")
                nc.sync.dma_start(x_sb, x8t.ap()[:, t])
                ktab = tab_pool.tile([128, 128], BF16, tag="ktab")
                nc.scalar.dma_start(ktab, kropes.ap()[t])
                qtab = tab_pool.tile([128, 256], BF16, tag="qtab")
                nc.scalar.dma_start(qtab[0:64, :], qropes.ap()[t])
                nc.scalar.dma_start(qtab[64:128, :], qropes.ap()[t])
                qbr = tab_pool.tile([128, C], BF16, tag="qbr")
                nc.scalar.dma_start(qbr, qbrope.ap()[t])
                return (x_sb, ktab, qtab, qbr)

            for _t in range(3):
                prefetched_x[_t] = load_x(_t)

            # per-group weight DMAs so the first matmuls can start early
            for g in range(NG):
                nc.scalar.dma_start(wkv_sb[:, g], wkv8.ap()[:, g])
            for j in range(KC):
                nc.sync.dma_start(wq_sb[:, j], wq8.ap()[:, j])

            vb_full = const_pool.tile([128, C], F32)
            nc.sync.dma_start(vb_full, vb512.ap().broadcast_to([128, C]))
            pb_full = const_pool.tile([128, C], F32)
            nc.sync.dma_start(pb_full, pb18.ap().broadcast_to([128, C]))

            # persistent kvT accumulator: pairs 0-3 in bank 0 (cols 0:512),
            # pairs 4-5 in bank 1 (cols 512:768, rest junk)
            kvt_ps = kvt_ps_pool.tile([128, 1024], F32, tag="kvt", name="kvt")

            state = {}

            def p1_front(t):
                x_sb, ktab, qtab, qbr = prefetched_x.pop(t, None) or load_x(t)

                # ---- k|v: out[tok, col] += sum_c x^T[c,:].T @ Wkv[c, col]
                # DoubleRow slots: (xh_c, xl_c) x (Wh_c, Wh_c)  [exact x]
                # then (xh_c, xh_c+1) x (Wl_c, Wl_c+1)          [W residual]
                kv_ps = kv_ps_pool.tile([128, 1536], F32, tag="kv")
                for g in range(NG):
                    dst = kv_ps[:, g * 512 : (g + 1) * 512]
                    for c in range(KC):
                        nc.tensor.matmul(
                            dst,
                            x_sb[:, c, :, :],
                            wkv_sb[:, g, c, 0:2, :],
                            start=(c == 0),
                            stop=False,
                            perf_mode=DR,
                        )
                    for cp in range(3):
                        c = 2 * cp
                        nc.tensor.matmul(
                            dst,
                            x_sb[:, c : c + 2, 0, :],
                            wkv_sb[:, g, c : c + 2, 2, :],
                            start=False,
                            stop=(cp == 2),
                            perf_mode=DR,
                        )

                # ---- q^T: out[cq, tok] += Wq[c, cq].T @ x^T[c, tok]
                # DoubleRow slots pair adjacent chunks (hi parts only);
                # qrot comes from a partition-swap DMA later, and the q bias
                # enters via the precomputed qbrope table.
                qq_ps = qq_ps_pool.tile([128, 1536], F32, tag="qq")
                for m in range(KC):
                    dst = qq_ps[:, m * 128 : (m + 1) * 128]
                    for j in range(3):
                        nc.tensor.matmul(
                            dst,
                            wq_sb[:, 2 * j : 2 * j + 2, m * 128 : (m + 1) * 128],
                            x_sb[:, 2 * j : 2 * j + 2, 0, :],
                            start=(m in (0, 4) and j == 0),
                            stop=(j == 2),
                            perf_mode=DR,
                        )
                state[t] = (kv_ps, qq_ps, ktab, qtab, qbr)

            def p1_back(t):
                kv_ps, qq_ps, ktab, qtab, qbr = state.pop(t)
                # ACT evicts PSUM -> SBUF bf16 fast (frees banks for the next
                # tile); DVE table-muls then run 2x from all-bf16 SBUF
                k_sb = work_pool.tile([128, C], BF16, tag="ksb")
                nc.scalar.copy(k_sb, kv_ps[:, 0:768])
                v_sb = work_pool.tile([128, C], BF16, tag="v")
                nc.scalar.copy(v_sb, kv_ps[:, 768:1536])
                nc.gpsimd.tensor_add(v_sb, v_sb, vb_full)
                qt_sb = work_pool.tile([128, C], BF16, tag="qt")
                nc.scalar.copy(qt_sb, qq_ps[:, 0:768])
                # qrot^T = partition-pair swap of q^T (engines cannot cross
                # partitions; a strided SBUF->SBUF DMA can)
                qrt_sb = work_pool.tile([128, C], BF16, tag="qrt")
                qtv = qt_sb.rearrange("(i two) f -> i two f", two=2)
                qrv = qrt_sb.rearrange("(i two) f -> i two f", two=2)
                nc.sync.dma_start(qrv[:, 0], qtv[:, 1])
                nc.sync.dma_start(qrv[:, 1], qtv[:, 0])
                # k-side rope (token-major); tables carry 1/SS
                ck = ktab[:, 0:64].unsqueeze(1).broadcast_to([128, H, 64])
                ske = ktab[:, 64:96].unsqueeze(1).broadcast_to([128, H, 32])
                sko = ktab[:, 96:128].unsqueeze(1).broadcast_to([128, H, 32])
                k1 = work_pool.tile([128, C], BF16, tag="k1")
                nc.vector.tensor_mul(
                    k1.rearrange("p (h d) -> p h d", h=H),
                    k_sb.rearrange("p (h d) -> p h d", h=H),
                    ck,
                )
                k2 = work_pool.tile([128, C], BF16, tag="k2")
                k2p = k2.rearrange("p (h i two) -> p h i two", h=H, two=2)
                ksp = k_sb.rearrange("p (h i two) -> p h i two", h=H, two=2)
                nc.vector.tensor_mul(k2p[:, :, :, 0], ksp[:, :, :, 1], ske)
                nc.vector.tensor_mul(k2p[:, :, :, 1], ksp[:, :, :, 0], sko)
                # q^T rope muls (channel-major; same table for all 6 chunks)
                cq = qtab[:, 0:128].unsqueeze(1).broadcast_to([128, KC, 128])
                sq = qtab[:, 128:256].unsqueeze(1).broadcast_to([128, KC, 128])
                q1 = work_pool.tile([128, C], BF16, tag="q1")
                nc.vector.tensor_mul(
                    q1.rearrange("p (j n) -> p j n", j=KC),
                    qt_sb.rearrange("p (j n) -> p j n", j=KC),
                    cq,
                )
                q2 = work_pool.tile([128, C], BF16, tag="q2")
                nc.vector.tensor_mul(
                    q2.rearrange("p (j n) -> p j n", j=KC),
                    qrt_sb.rearrange("p (j n) -> p j n", j=KC),
                    sq,
                )
                nc.gpsimd.tensor_add(qrs[:, t, :], q1, q2)
                nc.vector.tensor_add(qrs[:, t, :], qrs[:, t, :], qbr)
                # kvT accumulation (bf16, contraction over the 128 tokens)
                for p in range(KC):
                    sl = slice(p * 128, (p + 1) * 128)
                    for ki, ksrc_sb in enumerate((k1, k2)):
                        nc.tensor.matmul(
                            kvt_ps[:, sl],
                            v_sb[:, sl],
                            ksrc_sb[:, sl],
                            start=(t == 0 and ki == 0 and p in (0, 4)),
                            stop=(t == NT - 1 and ki == 1 and p in (3, 5)),
                        )

            for t in range(NT + 1):
                if t < NT:
                    p1_front(t)
                if t == 4:
                    for j in range(KC):
                        nc.scalar.dma_start(pw_sb[:, j], pw_eff.ap()[:, j])
                if t >= 1:
                    p1_back(t - 1)

            # ---- mid: M[d, c] = sum_e kv[h, d, e] * pw_eff[(h,e), c] ----
            kvt_sb = wpool.tile([128, C], BF16)
            nc.vector.tensor_copy(kvt_sb, kvt_ps[:, 0:768])
            kvm = kv_ps_pool.tile([128, 1536], F32, tag="kv")
            qqm = qq_ps_pool.tile([128, 1536], F32, tag="qq")
            for p in range(KC):
                slot = (kvm, qqm)[p % 2][:, 0:768]
                for gi in range(3):
                    gs = slice(gi * 256, (gi + 1) * 256)
                    # pending-zero from start=True covers only the matmul's
                    # own partitions, so each head clears its bank itself
                    st = gi % 2 == 0
                    sp = gi % 2 == 1 or gi == 2
                    nc.tensor.matmul(
                        slot[0:64, gs],
                        kvt_sb[0:64, p * 128 : p * 128 + 64],
                        pw_sb[0:64, p, gs],
                        start=st,
                        stop=sp,
                        tile_position=(0, 0),
                    )
                    nc.tensor.matmul(
                        slot[64:128, gs],
                        kvt_sb[64:128, p * 128 + 64 : p * 128 + 128],
                        pw_sb[64:128, p, gs],
                        start=st,
                        stop=sp,
                        tile_position=(64, 64),
                    )
                nc.scalar.copy(m_sb[:, p, 0:512], slot[:, 0:512])
                nc.scalar.copy(m_sb[:, p, 512:768], slot[:, 512:768])

            # ---- pass 2: out[tok, c] = qr^T.T @ M + pb  (bf16) ----
            p2_state = {}

            def p2_front(t):
                slot = (kvm, qqm, kvt_ps)[t % 3][:, 0:768]
                for gofs, glen in ((0, 512), (512, 256)):
                    dst = slot[:, gofs : gofs + glen]
                    for j in range(KC):
                        nc.tensor.matmul(
                            dst,
                            qrs[:, t, j * 128 : (j + 1) * 128],
                            m_sb[:, j, gofs : gofs + glen],
                            start=(j == 0),
                            stop=(j == KC - 1),
                        )
                p2_state[t] = slot

            def p2_back(t):
                slot = p2_state.pop(t)
                o_sb = out_pool.tile([128, C], BF16, tag="osb")
                nc.vector.tensor_add(o_sb, slot, pb_full)
                nc.sync.dma_start(out.ap()[t * 128 : (t + 1) * 128, :], o_sb)

            for t in range(NT + 1):
                if t < NT:
                    p2_front(t)
                if t >= 1:
                    p2_back(t - 1)

    nc.compile()
    return nc


def _prep_inputs(x, rope, qkv_w, q_bias, v_bias, proj_w, proj_b):
    f = np.float32

    sin = rope[:, :HD].astype(f)
    cos = rope[:, HD:].astype(f)
    cfull = np.zeros((NPAD, HD), f)
    cfull[0] = 1.0
    cfull[1:N] = cos
    sfull = np.zeros((NPAD, HD), f)
    sfull[1:N] = sin

    # k tables (token-major): ck | ske | sko, all carrying 1/SS
    kro = np.zeros((NPAD, 128), f)
    kro[:, 0:64] = cfull / SS
    kro[:, 64:96] = -sfull[:, 0::2] / SS
    kro[:, 96:128] = sfull[:, 1::2] / SS
    kropes = np.ascontiguousarray(kro.reshape(NT, 128, 128).astype(NPBF))

    # q tables (channel-major, transposed): cq^T | sq_signed^T
    sgn = np.tile(np.array([-1.0, 1.0], f), HD // 2)
    qro = np.zeros((NT, 64, 256), f)  # cast to bf16 below
    for t in range(NT):
        qro[t, :, 0:128] = cfull[t * 128 : (t + 1) * 128].T / SS
        qro[t, :, 128:256] = (sfull[t * 128 : (t + 1) * 128] * sgn[None, :HD]).T / SS

    wt = np.ascontiguousarray(qkv_w.T.astype(f))  # [C, 3C]
    Wq, Wkv = wt[:, :C], wt[:, C:]
    perm = np.arange(C).reshape(-1, 2)[:, ::-1].reshape(-1)

    kvh = (Wkv * SW).astype(NPF8)
    kvl = ((Wkv * SW) - kvh.astype(f)).astype(NPF8)
    # [128, NG, KC, 3, 512]: (hi, hi, lo)
    wkv8 = np.empty((128, NG, KC, 3, 512), NPF8)
    hi4 = kvh.reshape(KC, 128, NG, 512)
    lo4 = kvl.reshape(KC, 128, NG, 512)
    wkv8[:, :, :, 0, :] = hi4.transpose(1, 2, 0, 3)
    wkv8[:, :, :, 1, :] = hi4.transpose(1, 2, 0, 3)
    wkv8[:, :, :, 2, :] = lo4.transpose(1, 2, 0, 3)

    def wq_pack(W):
        w8 = (W * SW).astype(NPF8)
        return np.ascontiguousarray(w8.reshape(KC, 128, C).transpose(1, 0, 2))

    # rope-transformed q-bias term: rope(q + qb) = rope(q) + qb*c + rot(qb)*s
    qb = q_bias.astype(f)
    ctok = np.tile(cfull, (1, H))          # [NPAD, C]
    stok = np.tile(sfull, (1, H))
    sgn_c = np.tile(np.array([-1.0, 1.0], f), C // 2)
    qb_term = qb[None, :] * ctok + qb[perm][None, :] * (stok * sgn_c[None, :])
    qbrope_t = np.ascontiguousarray(
        qb_term.T.reshape(KC, 128, NT, 128).transpose(2, 1, 0, 3)
        .reshape(NT, 128, C).astype(NPBF)
    )

    pw = proj_w.T.astype(f) * (OS / (HD * N) / SS)
    pw_eff = np.ascontiguousarray(
        pw.reshape(KC, 128, C).transpose(1, 0, 2).astype(NPBF)
    )

    common = dict(
        wkv8=np.ascontiguousarray(wkv8),
        wq8=wq_pack(Wq),
        qbrope=qbrope_t,
        kropes=kropes,
        qropes=np.ascontiguousarray(qro.astype(NPBF)),
        vb512=np.ascontiguousarray(v_bias.astype(f)[None, :] * SS),
        pb18=np.ascontiguousarray(proj_b.astype(f)[None, :] * OS),
        pw_eff=pw_eff,
    )

    in_maps = []
    for b in range(B):
        xs = np.zeros((NPAD, C), f)
        xs[:N] = x[b] * SX
        xh = xs.astype(NPF8)
        xl = (xs - xh.astype(f)).astype(NPF8)
        stacked = np.stack([xh, xl], axis=0).reshape(2, NT, 128, KC, 128)
        m = dict(common)
        m["x8t"] = np.ascontiguousarray(stacked.transpose(4, 1, 3, 0, 2))
        in_maps.append(m)
    return in_maps


def kernel(x, rope, qkv_w, q_bias, v_bias, proj_w, proj_b, _trace=False):
    x = np.asarray(x, dtype=np.float32)
    rope = np.asarray(rope, dtype=np.float32)
    qkv_w = np.asarray(qkv_w, dtype=np.float32)
    q_bias = np.asarray(q_bias, dtype=np.float32)
    v_bias = np.asarray(v_bias, dtype=np.float32)
    proj_w = np.asarray(proj_w, dtype=np.float32)
    proj_b = np.asarray(proj_b, dtype=np.float32)
    if "nc" not in _CACHE:
        _CACHE["nc"] = _build_nc()
    nc = _CACHE["nc"]
    in_maps = _prep_inputs(x, rope, qkv_w, q_bias, v_bias, proj_w, proj_b)
    res = run_bass_kernel_spmd(nc, in_maps, core_ids=list(range(B)), trace=_trace)
    out = np.stack(
        [res.results[b]["out"][:N].astype(np.float32) for b in range(B)], axis=0
    )
    if _trace:
        _CACHE["last_result"] = res
    return out * np.float32(1.0 / OS)
